# revision 1
# baseline (speedup 1.0000x reference)
"""BinaryLayerWrapper (sync-BN + sign + binarized 3x3 conv) on 8 TRN2 cores.

Strategy (data-parallel, per sharding hint):
  - shard batch B=32 -> 4 images per core; conv weights replicated
  - phase A: stream x shard to SBUF (kept resident), per-channel partial
    sums sum(x), sum(x^2) over local batch+space trailing the DMA stream
  - sync-BN all-reduce of the [128,4] partial stats across the 8 cores
    via collective_compute (a 3-stage XOR-hypercube remote-DMA exchange
    is implemented behind USE_RDMA=1, but raw SWDGE remote descriptors
    hang on the axon fake_nrt runtime, so the collective is the default).
    Single-core builds model the exchange as the same local DRAM
    round-trip the baseline used (SBUF->DRAM, DRAM->DRAM, DRAM->SBUF).
  - per-channel a = gamma*rsqrt(var+eps), b = beta - mean*a
  - phase C: xb = Sign(a*x+b) in fp8 written into zero-padded 58x58
    planes; 3x3 conv = 9 fp8 DoubleRow accumulated matmuls per output
    tile (N=464 = 8 output rows x 58 padded cols), then scale by alpha
    and DMA the valid interior out.

The conv math is exact: xb is +-1 (exact in fp8e4m3), weights are
sign(w)/2 = +-0.5 (exact in fp8; the missing 2x is folded into alpha),
products accumulate in fp32 PSUM exactly.

Schedule notes (engine FIFOs execute in emission order, so emission is
chronological per engine):
  - weight DMAs are gated on the end of the x stream via tiny token
    writes, so they cannot displace x bytes (which gate sync-BN) on the
    shared DMA resource; they run during the allreduce+coef window
  - weights transpose directly from f32 (PE), and the PSUM->SBUF drain
    applies sign via one (w>=0)-0.5 tensor_scalar, so no activation-
    engine time is spent on weights: ACT does only stats, sqrt and the
    x sign passes
  - a dummy Sqrt activation at t=0 pins the act-func table that holds
    {sqrt, sign, copy, abs}, avoiding a 1.3us mid-kernel table reload
  - x-sign chunks are split so a conv tile at h0 only depends on sign
    chunks covering image rows <= h0+9 (matmul read spans bleed 2 cols
    into the next row), keeping the conv start fine-grained
  - conv drains alternate DVE/gpsimd; first drains go to DVE interleaved
    with the alpha reduces in ready-order; 6 PSUM banks absorb slack
  - discarded transposes gated on streaming scratch pace the PE through
    the load and bridge phases so the conv starts at full clock
"""

import os
from contextlib import ExitStack

import numpy as np

from concourse import bacc, bass, masks, mybir, tile
from concourse.bass_utils import run_bass_kernel_spmd

F32 = mybir.dt.float32
BF16 = mybir.dt.bfloat16
FP8 = mybir.dt.float8e4

N_CORES = 8
B_LOC = 4          # images per core (32 / 8)
C = 256            # channels (in == out)
KC = 2             # 128-partition channel chunks
H = W = 56
PIX = H * W        # 3136
WP = W + 2         # 58 padded width
PLANE = WP * (H + 2)          # 58*58 = 3364
XBP_LEN = PLANE + 2           # +1 lead pad so all tap offsets are >= 0
PLANE_PAD = 3376              # XBP_LEN rounded to 16 (fp8 DoubleRow Ko step)
R = 8                         # output rows per matmul tile (N=464, 1 PSUM bank)
NF = R * WP                   # 464 matmul free dim
N_TOTAL = 32 * PIX            # full-batch elements per channel (sync-BN)

# sync-BN exchange: XOR-hypercube remote DMAs (1) vs collective_compute (0).
# Raw RDMA descriptors are rejected by the axon fake_nrt runtime (even a
# self-loopback hangs), so the collective path is the default.
USE_RDMA = os.environ.get("USE_RDMA", "0") == "1"


def build_program(num_devices: int = N_CORES, cc: bool = True,
                  stage: int = 3) -> bass.Bass:
    nc = bacc.Bacc("TRN2", target_bir_lowering=False, debug=False,
                   num_devices=num_devices)
    nc._use_cc = cc
    nc._cc_devices = num_devices
    nc._stage = stage

    x = nc.dram_tensor("x", [B_LOC, C, H, W], F32, kind="ExternalInput").ap()
    w = nc.dram_tensor("weight", [C, C, 3, 3], F32, kind="ExternalInput").ap()
    gamma = nc.dram_tensor("gamma", [C], F32, kind="ExternalInput").ap()
    beta = nc.dram_tensor("beta", [C], F32, kind="ExternalInput").ap()
    y = nc.dram_tensor("y", [B_LOC, C, H, W], F32, kind="ExternalOutput").ap()

    nc._rdma_wait_patches = []
    with tile.TileContext(nc) as tc:
        _body(tc, y, x, w, gamma, beta)
    # The tile scheduler's single-core sim cannot observe remote semaphore
    # increments, so the receive-side folds are emitted without the remote
    # wait and the real semaphore waits are appended here, after scheduling
    # (extra waits can only delay the instruction, never break the schedule).
    for inst, sem, val in nc._rdma_wait_patches:
        si = inst.sync_info or mybir.SyncInfo(on_wait=[], on_update=[])
        nw = mybir.SyncWait(sync_type="semaphore", id=sem.num,
                            ant_name=sem.name, wait_mode="sem-ge-imm",
                            wait_value=val, wait_reg=None)
        inst.sync_info = mybir.SyncInfo(on_wait=list(si.on_wait) + [nw],
                                        on_update=list(si.on_update))
    nc.compile()
    return nc


def _body(tc: tile.TileContext, y, x, w, gamma, beta):
    nc = tc.nc
    add = mybir.AluOpType.add
    mult = mybir.AluOpType.mult
    AF = mybir.ActivationFunctionType
    n_dev = nc._cc_devices
    multi = nc._use_cc and n_dev > 1
    rdma = multi and USE_RDMA
    n_stages = max(1, (n_dev - 1).bit_length()) if rdma else 0

    with (
        tc.tile_pool(name="singles", bufs=1) as singles,
        tc.tile_pool(name="wsbuf", bufs=1) as wspool,
        tc.tile_pool(name="xres", bufs=1) as xpool,
        tc.tile_pool(name="stage", bufs=8) as stpool,
        tc.tile_pool(name="xbp", bufs=1) as xbpool,
        tc.tile_pool(name="dram", bufs=1, space="DRAM") as dram,
    ):
        identity = singles.tile([128, 128], BF16, tag="identity")
        masks.make_identity(nc, identity[:])
        identity8 = singles.tile([128, 128], FP8, tag="identity8")
        masks.make_identity(nc, identity8[:])

        # pin the {sqrt, sign, copy, abs} act table before any other
        # activation so it is loaded exactly once, at t=0
        actpin = singles.tile([128, 2], F32, tag="actpin")
        nc.gpsimd.memset(actpin[:, 0:1], 1.0)
        nc.scalar.activation(actpin[:, 1:2], actpin[:, 0:1], AF.Sqrt)

        gb = singles.tile([128, 4], F32, tag="gb")  # gamma k0,k1 | beta k0,k1
        g2 = singles.tile([128, 2], F32, tag="g2")  # gamma^2 per k

        NCH = 10  # stat chunks per k-chunk (3 images x 2 halves + 4 quarters)
        psum_parts = singles.tile([128, KC * NCH], F32, tag="psum_parts")
        psq_parts = singles.tile([128, KC * NCH], F32, tag="psq_parts")
        stats_local = singles.tile([128, 4], F32, tag="stats_local")
        # rx slots for the hypercube exchange + running partials
        rx = singles.tile([128, 12], F32, tag="rx")
        parts = singles.tile([128, 8], F32, tag="parts")  # p1 | p2
        gstats = singles.tile([128, 4], F32, tag="gstats")
        alpha_parts = singles.tile([128, 4], F32, tag="alpha_parts")
        alpha = singles.tile([128, 2], F32, tag="alpha")
        coefs = singles.tile([128, 12], F32, tag="coefs")
        ab = singles.tile([128, 4], F32, tag="ab")  # a k0,k1 | b k0,k1
        junk = singles.tile([128, 4], F32, tag="junk")
        bridge = singles.tile([128, 128], BF16, tag="bridge")

        xs = [[xpool.tile([128, PIX], F32, tag=f"xs{b}_{k}", name=f"xs{b}_{k}")
               for k in range(KC)] for b in range(B_LOC)]
        # per-oc fp8 weights, layout [cin_within_k, (k, tap, cout)]
        ws = [wspool.tile([128, KC * 9 * 128], FP8, tag=f"ws{oc}",
                          name=f"ws{oc}") for oc in range(2)]
        xbp = [xbpool.tile([128, KC * PLANE_PAD], FP8, tag=f"xbp{b}",
                           name=f"xbp{b}") for b in range(B_LOC)]

        if rdma:
            rsems = [nc.alloc_semaphore(name=f"bn_rx{s}")
                     for s in range(n_stages)]
            lsem = nc.alloc_semaphore(name="bn_tx")
            for s in rsems:
                nc.gpsimd.sem_clear(s)
            nc.gpsimd.sem_clear(lsem)

        # zero the halo borders (interior fully overwritten by the sign
        # pass; inter-plane alignment gap never read); DVE+Pool split
        def memset_borders(eng, t, base):
            eng.memset(t[:, base:base + 1], 0.0)
            eng.memset(t[:, base + 1:base + 1 + WP], 0.0)
            eng.memset(t[:, base + 1 + 57 * WP:base + 1 + 57 * WP + WP], 0.0)
            side = (t[:, base + 1 + WP:base + 1 + 57 * WP]
                    .rearrange("p (h w) -> p h w", w=WP))
            eng.memset(side[:, :, 0:1], 0.0)
            eng.memset(side[:, :, WP - 1:WP], 0.0)
            eng.memset(t[:, base + 1 + PLANE:base + 1 + PLANE + 1], 0.0)

        for b in range(B_LOC):
            for k in range(KC):
                eng = nc.vector if (b * KC + k) % 2 == 0 else nc.gpsimd
                memset_borders(eng, xbp[b], k * PLANE_PAD)

        # hypercube exchange descriptors, prepared early (data is read at
        # trigger time); stage s sends the running partial to tpb^(2^s)
        def stage_src(s):
            return stats_local[:] if s == 0 else parts[:, (s - 1) * 4:s * 4]

        if rdma:
            for s in range(n_stages):
                delta = 1 << s
                slot = 4 if (delta & 4) else 0  # cross-die needs slots 4-7
                rdests = [None] * 8
                rdests[slot] = (0, delta)
                nc.gpsimd.remote_dma_broadcast(
                    out_ap=rx[:, s * 4:s * 4 + 4], in_ap=stage_src(s),
                    remote_sem=rsems[s], local_sem=lsem, rdests=rdests)

        with (
            tc.tile_pool(name="wraw", bufs=1) as wraw_pool,
            tc.tile_pool(name="scr", bufs=2) as scr,
            tc.tile_pool(name="scrb", bufs=2) as scrb,
        ):
            psum_stack = ExitStack()
            wm_psum = psum_stack.enter_context(
                tc.tile_pool(name="wmps", bufs=1, space="PSUM"))
            tp_psum = psum_stack.enter_context(
                tc.tile_pool(name="tpps", bufs=2, space="PSUM"))
            cpsum = psum_stack.enter_context(
                tc.tile_pool(name="cpsum", bufs=5, space="PSUM"))

            def warm(src):
                # discarded transpose paces PE (p-state keep-warm)
                wt = wm_psum.tile([128, 128], BF16, tag="warm", name="warm")
                nc.tensor.transpose(wt[:], src, identity[:])

            # ---- phase A: x stream + trailing stats; last image in
            # quarter tiles so the post-stream stat tail is short ----
            HPIX = PIX // 2
            QPIX = PIX // 4
            chunks = []
            for b in range(B_LOC - 1):
                for k in range(KC):
                    for hf in range(2):
                        chunks.append((b, k, hf * HPIX, (hf + 1) * HPIX,
                                       k * NCH + b * 2 + hf))
            for k in range(KC):
                for q in range(4):
                    chunks.append((B_LOC - 1, k, q * QPIX, (q + 1) * QPIX,
                                   k * NCH + 6 + q))
            for (b, k, lo, hi, col) in chunks:
                nc.sync.dma_start(
                    out=xs[b][k][:, lo:hi],
                    in_=x[b, k * 128:(k + 1) * 128]
                    .rearrange("c h w -> c (h w)")[:, lo:hi])
                n = hi - lo
                xsl = xs[b][k][:, lo:hi]
                sa = scr.tile([128, HPIX], BF16, tag="scr_a", name="scr_a")
                nc.scalar.activation(sa[:, 0:n], xsl, AF.Copy,
                                     accum_out=psum_parts[:, col:col + 1])
                sb = scrb.tile([128, HPIX], BF16, tag="scr_b", name="scr_b")
                nc.vector.scalar_tensor_tensor(
                    out=sb[:, 0:n], in0=xsl, scalar=1.0, in1=xsl,
                    op0=mult, op1=mult,
                    accum_out=psq_parts[:, col:col + 1])
                warm(sa[:, 0:128])

            # gamma/beta after the x stream so they don't delay it
            nc.sync.dma_start(out=gb[:, 0:2],
                              in_=gamma.rearrange("(k p) -> p k", p=128))
            nc.sync.dma_start(out=gb[:, 2:4],
                              in_=beta.rearrange("(k p) -> p k", p=128))
            nc.gpsimd.tensor_tensor(out=g2[:], in0=gb[:, 0:2], in1=gb[:, 0:2],
                                    op=mult)

            # ---- finalize local stats ----
            nc.vector.tensor_reduce(
                out=stats_local[:, 0:2],
                in_=psum_parts[:].rearrange("p (k n) -> p k n", k=KC),
                axis=mybir.AxisListType.X, op=add)
            nc.vector.tensor_reduce(
                out=stats_local[:, 2:4],
                in_=psq_parts[:].rearrange("p (k n) -> p k n", k=KC),
                axis=mybir.AxisListType.X, op=add)

            # PE pacing through the bridge: tiny copies of allreduce/coef
            # products into the bridge tile give freshly-written warm gates
            def bridge_warm(i, gate):
                nc.vector.tensor_copy(bridge[:, 4 * i:4 * i + 4], gate)
                warm(bridge[:, 0:128])

            # ---- sync-BN exchange ----
            if rdma:
                # per stage: a Pool read of the stage source orders the
                # trigger after the data write; DVE waits the remote sem
                # (+2 per arrived send) then folds the received slot in
                for s in range(n_stages):
                    acc_in = stage_src(s)
                    acc_out = (parts[:, s * 4:s * 4 + 4] if s < n_stages - 1
                               else gstats[:])
                    nc.gpsimd.tensor_copy(junk[:, s:s + 1], acc_in[:, 0:1])
                    nc.gpsimd.trigger_dma(1)
                    bi = nc.vector.tensor_tensor(
                        out=acc_out, in0=acc_in,
                        in1=rx[:, s * 4:s * 4 + 4], op=add)
                    nc._rdma_wait_patches.append(
                        (bi.ins, rsems[s], 2 * (s + 1)))
                    if s == 0:
                        bridge_warm(0, acc_out[:, 0:4])
            elif multi:
                ccin = dram.tile([128, 4], F32, tag="ccin", name="ccin")
                ccout = dram.tile([128, 4], F32, tag="ccout", name="ccout")
                nc.sync.dma_start(out=ccin[:], in_=stats_local[:])
                nc.gpsimd.collective_compute(
                    "AllReduce", add,
                    replica_groups=[list(range(n_dev))],
                    ins=[ccin.opt()], outs=[ccout.opt()])
                nc.sync.dma_start(out=gstats[:], in_=ccout[:])
            elif os.environ.get("STANDIN_HOPS", "3") == "3":
                # single-core stand-in for the collective: the same local
                # DRAM round-trip the baseline modeled (SBUF->DRAM, a
                # DRAM->DRAM hop for the allreduce, DRAM->SBUF)
                ccin = dram.tile([128, 4], F32, tag="ccin", name="ccin")
                ccout = dram.tile([128, 4], F32, tag="ccout", name="ccout")
                nc.sync.dma_start(out=ccin[:], in_=stats_local[:])
                nc.sync.dma_start(out=ccout[:], in_=ccin[:])
                nc.sync.dma_start(out=gstats[:], in_=ccout[:])
                bridge_warm(0, gstats[:, 0:4])
            else:
                # 1-hop stand-in variant (local SBUF->SBUF exchange)
                nc.sync.dma_start(out=rx[:, 0:4], in_=stats_local[:])
                nc.vector.tensor_scalar_add(gstats[:], rx[:, 0:4], 0.0)
                bridge_warm(0, gstats[:, 0:4])

            # ---- weight DMA: token writes gated on the local stats keep
            # the 8 sub-chunks strictly after the x stream on the shared
            # DMA resource (the resource is granted in request order) ----
            wraws = []
            for oc in range(2):
                wraw = wraw_pool.tile([128, C * 9], F32, tag=f"wraw{oc}",
                                      name=f"wraw{oc}")
                wraws.append(wraw)
            for oc in range(2):
                for i in range(4):
                    nc.gpsimd.tensor_copy(wraws[oc][:, i * 576:i * 576 + 1],
                                          stats_local[:, 0:1])
            for oc in range(2):
                wsrc = w[oc * 128:(oc + 1) * 128].rearrange(
                    "o c kh kw -> o (c kh kw)")
                for i in range(4):
                    sl = slice(i * 576, (i + 1) * 576)
                    nc.sync.dma_start(out=wraws[oc][:, sl], in_=wsrc[:, sl])

            # ---- BN coefficients: a = sqrt(gamma^2 / (var+eps)),
            # b = beta - mean*a ----
            bridge_warm(1, gstats[:, 0:4])
            mean = coefs[:, 0:2]
            msq = coefs[:, 2:4]
            m2 = coefs[:, 4:6]
            var = coefs[:, 6:8]
            rec = coefs[:, 10:12]
            nc.vector.tensor_scalar_mul(coefs[:, 0:4], gstats[:],
                                        1.0 / N_TOTAL)
            nc.vector.tensor_tensor(out=m2, in0=mean, in1=mean, op=mult)
            nc.vector.scalar_tensor_tensor(
                out=var, in0=msq, scalar=1e-5, in1=m2,
                op0=add, op1=mybir.AluOpType.subtract)
            nc.vector.reciprocal(rec, var)
            for k in range(KC):
                nc.scalar.activation(ab[:, k:k + 1], rec[:, k:k + 1],
                                     AF.Sqrt, scale=g2[:, k:k + 1])
            nc.vector.tensor_tensor(out=coefs[:, 4:6], in0=mean,
                                    in1=ab[:, 0:2], op=mult)
            nc.vector.tensor_tensor(out=ab[:, 2:4], in0=gb[:, 2:4],
                                    in1=coefs[:, 4:6],
                                    op=mybir.AluOpType.subtract)
            bridge_warm(2, ab[:, 0:4])
            # WAR gate: reading alpha_parts here makes the bulk alpha
            # reduces wait for ab, so the scheduler cannot hoist them
            # ahead of the sign-gating coefficient chain on DVE
            nc.vector.tensor_tensor(out=junk[:, 0:4], in0=alpha_parts[:],
                                    in1=ab[:], op=add)

            # ---- weight prep, per (oc,k) chunk as its DMA lands: Pool
            # turns w into sign(w)/2 = +-0.5 fp8 in SBUF (alpha carries
            # the 2x; gpsimd cannot touch PSUM), PE transposes the fp8
            # into one [128,1152] PSUM tile per (oc,k).  Only DVE/ACT can
            # read PSUM back: DVE drains oc0 (it gates the conv start),
            # ACT drains oc1 after the image-0 signs (needed ~6us later)
            w05s = []
            tgroups = [(0, 4), (4, 8), (8, 9)]
            for oc in range(2):
                w05 = wraw_pool.tile([128, C * 9], BF16, tag=f"w05_{oc}",
                                     name=f"w05_{oc}")
                w05s.append(w05)
            for oc in range(2):
                for k in range(KC):
                    sl = slice(k * 1152, (k + 1) * 1152)
                    nc.gpsimd.tensor_scalar(
                        out=w05s[oc][:, sl], in0=wraws[oc][:, sl],
                        scalar1=0.0, scalar2=0.5,
                        op0=mybir.AluOpType.is_ge,
                        op1=mybir.AluOpType.subtract)

            def wprep(oc, k, drain):
                # transpose one (oc,k) chunk into PSUM tap-groups and
                # drain them to the fp8 lhsT tile on the given engine
                w3 = w05s[oc][:].rearrange("o (c t) -> o c t", t=9)
                for (t0, t1) in tgroups:
                    pool = tp_psum if t1 - t0 == 4 else wm_psum
                    pst = pool.tile([128, (t1 - t0) * 128], BF16,
                                    tag="warm" if t1 - t0 == 1 else "tp4",
                                    name="tp")
                    for t in range(t0, t1):
                        nc.tensor.transpose(
                            pst[:, (t - t0) * 128:(t - t0 + 1) * 128],
                            w3[:, k * 128:(k + 1) * 128, t],
                            identity[:])
                    dst = ws[oc][:, (k * 9 + t0) * 128:(k * 9 + t1) * 128]
                    if drain == "dve":
                        nc.vector.tensor_copy(dst, pst[:])
                    else:
                        nc.scalar.activation(dst, pst[:], AF.Copy)

            # oc0 gates the conv start: prep it now, drains on DVE; oc1's
            # drains go to ACT after the image-0 signs (needed ~6us later)
            wprep(0, 0, "dve")
            wprep(0, 1, "dve")

            # ---- phase C: binarize into padded planes + conv ----
            def emit_sign(b, k, r0, r1):
                base = k * PLANE_PAD
                nr = r1 - r0
                lo = base + 1 + (1 + r0) * WP + 1
                interior = (xbp[b][:, lo:lo + (nr + 1) * WP]
                            .rearrange("p (h w) -> p h w", w=WP)[:, 0:nr, 0:W])
                nc.scalar.activation(
                    interior,
                    xs[b][k][:].rearrange("p (h w) -> p h w", w=W)[:, r0:r1, :],
                    AF.Sign,
                    bias=ab[:, 2 + k:3 + k], scale=ab[:, k:k + 1])

            splits = {0: ((0, 28), (28, H))}
            for b in range(1, B_LOC):
                splits[b] = ((0, 29), (29, H))
            # image 0 signs up front (they gate the conv start); later
            # images' sign pairs are interleaved into the conv emission so
            # the ACT queue stays chronological with its drains
            for rr in splits[0]:
                for k in range(KC):
                    emit_sign(0, k, rr[0], rr[1])
            wprep(1, 0, "act")
            wprep(1, 1, "act")
            # oc1 weight prep and later images' sign pairs interleave into
            # the conv emission so the PE/ACT FIFOs stay chronological
            sign_at = {}
            for b in range(1, B_LOC):
                base_ti = 14 * (b - 1)
                sign_at[base_ti + 2] = (b, splits[b][0])
                sign_at[base_ti + 7] = (b, splits[b][1])


            # alpha = 2 * mean|w| per oc (2x compensates the +-0.5
            # weights); pieces gated per (oc,k) DMA chunk
            def alpha_piece(oc, k):
                nc.vector.tensor_reduce(
                    out=alpha_parts[:, oc * 2 + k:oc * 2 + k + 1],
                    in_=wraws[oc][:, k * 1152:(k + 1) * 1152],
                    axis=mybir.AxisListType.X, op=add,
                    apply_absolute_value=True)

            def alpha_comb(oc):
                nc.vector.tensor_reduce(
                    out=coefs[:, 8 + oc:9 + oc],
                    in_=alpha_parts[:, oc * 2:oc * 2 + 2],
                    axis=mybir.AxisListType.X, op=add)
                nc.vector.tensor_scalar_mul(alpha[:, oc:oc + 1],
                                            coefs[:, 8 + oc:9 + oc],
                                            2.0 / (C * 9))

            alpha_piece(0, 0)
            alpha_piece(0, 1)
            alpha_comb(0)

            # conv tiles; image 0 runs all oc=0 first (oc=1 lhsT tiles
            # land later), later images interleave
            tiles = []
            for oc in range(2):
                for h0 in range(0, H, R):
                    tiles.append((0, h0, oc))
            for b in range(1, B_LOC):
                for h0 in range(0, H, R):
                    for oc in range(2):
                        tiles.append((b, h0, oc))

            if nc._stage <= 2:
                nc.sync.dma_start(out=y[0, 0:128, 0, 0:4], in_=ab[:])
                return

            for ti, (b, h0, oc) in enumerate(tiles):
                if ti in sign_at:
                    sb_, rr = sign_at[ti]
                    for k in range(KC):
                        emit_sign(sb_, k, rr[0], rr[1])
                acc = cpsum.tile([128, NF], F32, tag="acc", name="acc")
                xv = xbp[b][:].rearrange("p (i l) -> p i l", l=PLANE_PAD)
                lhsT = ws[oc][:].rearrange("p (i t m) -> p i t m", i=KC, m=128)
                for tap in range(9):
                    dh, dw = tap // 3, tap % 3
                    off = (h0 + dh) * WP + dw
                    nc.tensor.matmul(
                        acc[:], lhsT[:, :, tap, :], xv[:, :, off:off + NF],
                        start=(tap == 0), stop=(tap == 8),
                        perf_mode=mybir.MatmulPerfMode.DoubleRow)
                stage = stpool.tile([128, R, W], F32, tag="stage", name="stage")
                accv = (acc[:].rearrange("p (h w) -> p h w", w=WP)[:, :, 1:1 + W])
                if ti == 2:
                    alpha_piece(1, 0)
                if ti == 4:
                    alpha_piece(1, 1)
                    alpha_comb(1)
                if ti < 8 or ti % 2 == 1:
                    nc.vector.tensor_scalar_mul(stage[:], accv,
                                                alpha[:, oc:oc + 1])
                else:
                    nc.scalar.activation(stage[:], accv, AF.Copy,
                                         scale=alpha[:, oc:oc + 1])
                nc.sync.dma_start(
                    out=y[b, oc * 128:(oc + 1) * 128, h0:h0 + R, :],
                    in_=stage[:])
            psum_stack.close()


def run_on_hw(x, weight, gamma, beta, **spmd_kwargs):
    nc = build_program()
    in_maps = []
    for i in range(N_CORES):
        in_maps.append({
            "x": np.ascontiguousarray(x[i * B_LOC:(i + 1) * B_LOC]),
            "weight": np.ascontiguousarray(weight),
            "gamma": np.ascontiguousarray(gamma),
            "beta": np.ascontiguousarray(beta),
        })
    return run_bass_kernel_spmd(nc, in_maps, core_ids=list(range(N_CORES)),
                                **spmd_kwargs)


def kernel(x: np.ndarray, weight: np.ndarray, gamma: np.ndarray,
           beta: np.ndarray) -> np.ndarray:
    # The first execution on a freshly-attached device occasionally reports
    # NRT_EXEC_UNIT_UNRECOVERABLE from residue of a prior process; an
    # immediate retry reliably succeeds.
    last_err = None
    for _ in range(3):
        try:
            res = run_on_hw(x, weight, gamma, beta)
            break
        except Exception as e:  # noqa: BLE001 - retry any transient runtime error
            last_err = e
    else:
        raise last_err
    out = np.concatenate([res.results[i]["y"] for i in range(N_CORES)], axis=0)
    return out.astype(np.float32)


if __name__ == "__main__":
    nc = build_program()
    print("build ok:", len(nc.inst_map), "instructions")

